# revision 40
# baseline (speedup 1.0000x reference)
"""Trainium2 Bass kernel for BilinearInteraction.

Reference math (B=2048, F=32 fields, D=64, P=496 field-pairs):
    for pair p=(i,j):  out[b,p,:] = (v_i @ W[p].T) * v_j
    v_i = feature_emb[:, i, :],  v_j = feature_emb[:, j, :]

Sharding: data-parallel over batch, 8 cores x 256 rows each; W replicated.
The fp32 output is 260MB (32.5MB/core) -> the kernel is output-write bound,
so the device writes bf16 (16.25MB/core) and the host upcasts; combined with
bf16 matmul operands the end-to-end relative error is ~3e-3, well inside the
2e-2 gate.

Per-core dataflow (all static, Tile-scheduled):
  - W is pre-transposed, cast to bf16 and packed host-side into
    wpack[128, 16384]: partitions 0:64 hold pairs 0..255 (cols p*64+e =
    W[p,e,d=partition]), partitions 64:128 hold pairs 256..495. Loaded as
    four resident [128,4096] tiles via column-sliced DMAs interleaved so
    the first compute stage's slice lands first.
  - featT[128, 5632] bf16 = per-field transposed features, the stationary
    matmul operand. A matmul requires lhsT/rhs to share a base partition,
    and pairs 0..255 (partitions 0:64) only ever use first-fields 0..9
    while pairs 256..495 (partitions 64:128) use 9..30 - so partitions
    0:64 hold fields 0..9 (col f*256+b) and partitions 64:128 hold fields
    9..30 (col (f-9)*256+b), one full-width DMA, no duplication.
  - featN[256, 2048] f32 = natural-layout features; the elementwise
    multiplier for consecutive pairs of one group is a contiguous slab.
  - Per (batch-half bc, output stage = 16..64 consecutive pairs; early
    stages are small so the output stream starts ~15us sooner): pairs
    grouped into "runs" (same first field, one 64-pair W block, <=16
    pairs). Each run: 1-2 matmuls [K=64,M=128]x[N<=512] into consecutive
    PSUM banks of one tile, then the PSUM x featN Hadamard product via
    one of two engine paths chosen to balance load (DVE TT from PSUM runs
    at ~95 elem/ns; GpSimd cannot read PSUM, so its path is ACT copy
    PSUM->SBUF f32 at ~95 then GpSimd TT at ~56; ACT/GpSimd are
    otherwise idle):
       path A (~65%): DVE  tensor_mul(psum_f32, featN_f32) -> stage bf16
       path C (~35%): ACT  copy psum -> tmp f32;
                      GPS  tensor_mul(tmp, featN_f32)      -> stage bf16
    Stage completes with one HWDGE DMA to the output row-block (the
    output lands directly in natural [b, p*64+e] layout); the very last
    stage drains per run to shorten the kernel tail.
"""

from itertools import combinations

import numpy as np

N_CORES = 8
B, F, D = 2048, 32, 64
P = 496
B_SH = B // N_CORES            # 256 batch rows per core
HALF = 256                     # pair index where the partition half flips
RUN = 16                       # max pairs per Hadamard op (2 PSUM banks)
GPS_FRAC = 0.35                # share of elements routed to the GpSimd path

# output stages as (pair_lo, pair_hi); first ones small to prime the pipe,
# last ones small to shorten the serial kernel tail
_BOUNDS0 = [0, 8, 16, 32, 64, 128, 192, 256, 320, 384, 448, 496]
_BOUNDS1 = [0, 64, 128, 192, 256, 320, 384, 448, 480, 496]
STAGES = {0: list(zip(_BOUNDS0[:-1], _BOUNDS0[1:])),
          1: list(zip(_BOUNDS1[:-1], _BOUNDS1[1:]))}

PAIRS = list(combinations(range(F), 2))

_NC_CACHE = {}


def _runs(lo, hi):
    """Runs of consecutive same-group pairs (<=RUN) in [lo,hi), not
    crossing 64-pair W-block boundaries."""
    runs = []
    p = lo
    while p < hi:
        i = PAIRS[p][0]
        e = p
        while (e + 1 < hi and PAIRS[e + 1][0] == i and (e + 1 - p) < RUN
               and (e + 1) % 64 != 0):
            e += 1
        runs.append((p, e - p + 1))
        p = e + 1
    return runs


def _build():
    import concourse.tile as tile
    from concourse import bacc, mybir

    F32 = mybir.dt.float32
    BF16 = mybir.dt.bfloat16
    nc = bacc.Bacc("TRN2", target_bir_lowering=False, debug=False,
                   enable_asserts=False, num_devices=N_CORES)

    wpack = nc.dram_tensor("wpack", [128, 4 * 4096], BF16, kind="ExternalInput").ap()
    featT = nc.dram_tensor("featT", [128, 22 * B_SH], BF16, kind="ExternalInput").ap()
    featN = nc.dram_tensor("featN", [B_SH, F * D], F32, kind="ExternalInput").ap()
    out = nc.dram_tensor("out", [B_SH, P * D], BF16, kind="ExternalOutput").ap()

    with tile.TileContext(nc) as tc:
        with (
            tc.tile_pool(name="win", bufs=1) as win,
            tc.tile_pool(name="feat", bufs=1) as feat,
            tc.tile_pool(name="stage", bufs=6) as stage_pool,
            tc.tile_pool(name="tmp", bufs=8) as tmp_pool,
            tc.tile_pool(name="psum", bufs=4, space="PSUM") as psum_pool,
        ):
            # resident input tiles ------------------------------------------------
            w = [win.tile([128, 4096], BF16, name=f"w{blk}", tag=f"w{blk}")
                 for blk in range(4)]
            ft = feat.tile([128, 22 * B_SH], BF16, name="ft", tag="ft")
            fn = [feat.tile([128, F * D], F32, name=f"fn{bc}", tag=f"fn{bc}")
                  for bc in range(2)]

            # issue order = first-compute order; fine slices first so the
            # pipeline primes fast (all on the scalar HWDGE ring; outputs
            # use the sync ring)
            nc.scalar.dma_start(ft[:, 0:512], featT[:, 0:512])
            nc.scalar.dma_start(w[0][:, 0:1024], wpack[:, 0:1024])
            nc.scalar.dma_start(w[0][:, 1024:2048], wpack[:, 1024:2048])
            nc.scalar.dma_start(fn[0][:, :], featN[0:128, :])
            nc.scalar.dma_start(ft[:, 512:2560], featT[:, 512:2560])
            nc.scalar.dma_start(w[0][:, 2048:4096], wpack[:, 2048:4096])
            nc.scalar.dma_start(w[1][:, :], wpack[:, 4096:8192])
            nc.scalar.dma_start(w[2][:, :], wpack[:, 8192:12288])
            nc.scalar.dma_start(ft[:, 2560:22 * B_SH], featT[:, 2560:22 * B_SH])
            nc.scalar.dma_start(w[3][:, :], wpack[:, 12288:16384])
            nc.scalar.dma_start(fn[1][:, :], featN[128:256, :])

            # compute + output ----------------------------------------------------
            el_tot = el_gps = 0
            for bc in range(2):
                stages = STAGES[bc]
                for si, (lo, hi) in enumerate(stages):
                    runs = _runs(lo, hi)
                    st = stage_pool.tile([128, (hi - lo) * D], BF16, tag="stage")
                    for ri, (p0, n) in enumerate(runs):
                        i, j0 = PAIRS[p0]
                        h = p0 // HALF
                        po = 64 * h
                        fcol = (i - 9 * h) * B_SH   # field col in ft's half
                        colbase = (p0 - h * HALF) * D
                        blk, bcol = colbase // 4096, colbase % 4096
                        if n <= 8:
                            ps = psum_pool.tile([128, 8 * D], F32, tag="ps8",
                                                bufs=2)
                        else:
                            ps = psum_pool.tile([128, RUN * D], F32, tag="ps",
                                                bufs=3)
                        for k in range(0, n, 8):
                            nk = min(8, n - k)
                            nc.tensor.matmul(
                                ps[:, k * D:(k + nk) * D],
                                lhsT=ft[po:po + 64,
                                        fcol + bc * 128:
                                        fcol + bc * 128 + 128],
                                rhs=w[blk][po:po + 64,
                                           bcol + k * D: bcol + (k + nk) * D],
                                start=True, stop=True,
                            )
                        st_sl = st[:, (p0 - lo) * D: (p0 - lo + n) * D]
                        fn_sl = fn[bc][:, j0 * D: (j0 + n) * D]
                        el_tot += n
                        if el_gps < GPS_FRAC * el_tot:
                            el_gps += n
                            tmp = tmp_pool.tile([128, RUN * D], F32, tag="tmp")
                            nc.scalar.copy(tmp[:, 0:n * D], ps[:, 0:n * D])
                            nc.gpsimd.tensor_mul(st_sl, tmp[:, 0:n * D], fn_sl)
                        else:
                            nc.vector.tensor_mul(st_sl, ps[:, 0:n * D], fn_sl)
                    nc.sync.dma_start(
                        out[bc * 128: bc * 128 + 128, lo * D: hi * D],
                        st[:, :])
    nc.compile()
    return nc


def _pack_inputs(feature_emb, W):
    import ml_dtypes

    BF = ml_dtypes.bfloat16
    feature_emb = np.ascontiguousarray(feature_emb, dtype=np.float32)
    W = np.ascontiguousarray(W, dtype=np.float32)
    Wt = W.transpose(0, 2, 1)                      # [P, d, e]
    wpack = np.zeros((128, 4 * 4096), dtype=BF)
    wpack[0:64, :] = Wt[0:HALF].transpose(1, 0, 2).reshape(64, HALF * D).astype(BF)
    wpack[64:128, 0:(P - HALF) * D] = (
        Wt[HALF:P].transpose(1, 0, 2).reshape(64, (P - HALF) * D).astype(BF))
    in_maps = []
    for c in range(N_CORES):
        shard = feature_emb[c * B_SH:(c + 1) * B_SH]         # [256, 32, 64]
        # [d, f, b] per-field transposed features
        ftT = shard.transpose(2, 1, 0).astype(BF)            # [64, 32, 256]
        featT = np.zeros((128, 22 * B_SH), dtype=BF)
        # partitions 0:64 <- fields 0..9 (first-fields of pairs 0..255)
        featT[0:64, 0:10 * B_SH] = ftT[:, 0:10].reshape(64, 10 * B_SH)
        # partitions 64:128 <- fields 9..30 (first-fields of pairs 256..495)
        featT[64:128, :] = ftT[:, 9:31].reshape(64, 22 * B_SH)
        in_maps.append({
            "wpack": wpack,
            "featT": featT,
            "featN": np.ascontiguousarray(shard.reshape(B_SH, F * D)),
        })
    return in_maps


def kernel(feature_emb, W, _trace=False):
    from concourse.bass_utils import run_bass_kernel_spmd

    if "nc" not in _NC_CACHE:
        _NC_CACHE["nc"] = _build()
    nc = _NC_CACHE["nc"]
    in_maps = _pack_inputs(feature_emb, W)
    res = run_bass_kernel_spmd(nc, in_maps, core_ids=list(range(N_CORES)),
                               trace=_trace)
    full = np.concatenate(
        [res.results[c]["out"].astype(np.float32) for c in range(N_CORES)], axis=0)
    out = full.reshape(B, P, D)
    if _trace:
        return out, res
    return out
